# revision 1
# baseline (speedup 1.0000x reference)
"""HQQ-compatible 4-bit quantized linear layer on 8 Trainium2 NeuronCores.

Problem: y = x @ W.T + bias where W = ((unpack4(W_q) - zero) * scale).reshape(8192, 8192)
  x: (64, 8192) f32; W_q: (32, 1048576) int32 (bytes, two nibbles packed);
  scale/zero: (1, 1048576) f32; bias: (8192,) f32.

Math per output element (OUT=IN=8192, GS=64, NG=2**20):
  W[o, i] = (Wu[gs, ng] - zero[ng]) * scale[ng],  gs = o // 128, ng = (o % 128)*8192 + i
  Wu[r, ng] = W_q[r, ng] >> 4 (r < 32) | W_q[r-32, ng] & 0xF (r >= 32).

Sharding (tensor-parallel over output features, by ng blocks):
  core m owns ng in [m*131072, (m+1)*131072)  <=>  (o % 128) in [m*16, m*16+16).
  scale/zero/W_q columns shard cleanly; x replicated; core m computes the 1024
  outputs o = gs*128 + m*16 + b (gs in [0,64), b in [0,16)).

Per-core device pipeline (linearity: y = sum x*sc*Wu - sum x*(sc*zero) + bias):
  - host splits W_q bytes into hi/lo nibble u8 arrays (pure bit-repacking; all
    arithmetic stays on device) laid out [chunk, p, (k_local, r, b)]
  - hi: HWDGE DMA u8 -> SBUF, ScalarE activation-copy casts u8 -> f16
    lo: SWDGE DMA casts u8 -> f16 in-flight (no engine time)
  - VectorE: one tensor_tensor mult per (nibble, 8-k chunk): f16 nibbles times
    scale broadcast over r (4-dim AP, b-minor step-1 keeps the 2x perf mode),
    writing the two halves of a [128, k, 1024] moving-operand buffer
  - TensorE: per k one N=1024 matmul (hi|lo) + one N=16 matmul (sc*zero term),
    all accumulating over the 64 k-tiles in PSUM
  - epilogue: y = psum - zero_term (broadcast over r) + bias, DMA out (64,1024) f32
"""

import ml_dtypes
import numpy as np

OUT = 8192
IN = 8192
GS = 64
NG = OUT * IN // GS  # 1048576
B = 64
NCORES = 8
NGC = NG // NCORES   # 131072 groups per core
BB = 16              # width of the (o % 128) block per core
KT = IN // 128       # 64 in-tiles of 128
CK = 4               # k-tiles per chunk
NCH = KT // CK       # 8 chunks

_CACHE = {}


def _build_nc():
    import concourse.bacc as bacc
    import concourse.mybir as mybir
    import concourse.tile as tile
    from concourse.alu_op_type import AluOpType

    f16 = mybir.dt.bfloat16
    f32 = mybir.dt.float32
    u8 = mybir.dt.uint8

    nc = bacc.Bacc(None, target_bir_lowering=False, debug=False)

    xt_d = nc.dram_tensor("xt", [128, KT * B], f16, kind="ExternalInput")
    hi_d = nc.dram_tensor("hi", [NCH, 128, CK * 512], u8, kind="ExternalInput")
    lo_d = nc.dram_tensor("lo", [NCH, 128, CK * 512], u8, kind="ExternalInput")
    sc_d = nc.dram_tensor("sc", [128, KT * BB], f16, kind="ExternalInput")
    sz_d = nc.dram_tensor("sz", [128, KT * BB], f16, kind="ExternalInput")
    bs_d = nc.dram_tensor("bs", [1, 1024], f32, kind="ExternalInput")
    y_d = nc.dram_tensor("y", [B, 1024], f32, kind="ExternalOutput")

    with tile.TileContext(nc) as tc:
        with (
            tc.tile_pool(name="const", bufs=1) as cpool,
            tc.tile_pool(name="wq", bufs=3) as wqpool,
            tc.tile_pool(name="nib", bufs=3) as nibpool,
            tc.tile_pool(name="ws", bufs=3) as wspool,
            tc.tile_pool(name="psum", bufs=1, space="PSUM") as pspool,
            tc.tile_pool(name="outp", bufs=1) as opool,
        ):
            xt_sb = cpool.tile([128, KT * B], f16)
            nc.sync.dma_start(out=xt_sb[:], in_=xt_d[:])
            sc_sb = cpool.tile([128, KT * BB], f16)
            nc.sync.dma_start(out=sc_sb[:], in_=sc_d[:])
            sz_sb = cpool.tile([128, KT * BB], f16)
            nc.sync.dma_start(out=sz_sb[:], in_=sz_d[:])
            bias_sb = cpool.tile([B, 1024], f32)
            nc.sync.dma_start(out=bias_sb[:], in_=bs_d[0:1, :].broadcast_to((B, 1024)))

            psW = pspool.tile([B, 1024], f32)   # (hi | lo) accumulated
            psC = pspool.tile([B, BB], f32)     # zero-term

            for c in range(NCH):
                # hi path: u8 DMA + ScalarE cast
                hi_u8 = wqpool.tile([128, CK * 512], u8, tag="hi_u8")
                nc.sync.dma_start(out=hi_u8[:], in_=hi_d[c])
                hi_f = nibpool.tile([128, CK * 512], f16, tag="hi_f")
                nc.scalar.activation(
                    out=hi_f[:], in_=hi_u8[:],
                    func=mybir.ActivationFunctionType.Copy, scale=1.0,
                )
                # lo path: SWDGE cast-DMA straight to f16
                lo_f = nibpool.tile([128, CK * 512], f16, tag="lo_f")
                nc.gpsimd.dma_start(out=lo_f[:], in_=lo_d[c])

                ws = wspool.tile([128, CK * 1024], f16, tag="ws")
                ws4 = ws[:].rearrange("p (k n) -> p k n", n=1024)
                sc_bc = (
                    sc_sb[:, c * CK * BB : (c + 1) * CK * BB]
                    .rearrange("p (k b) -> p k b", b=BB)
                    .unsqueeze(2)
                    .broadcast_to((128, CK, 32, BB))
                )
                nc.vector.tensor_tensor(
                    out=ws4[:, :, 0:512].rearrange("p k (r b) -> p k r b", b=BB),
                    in0=hi_f[:].rearrange("p (k r b) -> p k r b", k=CK, b=BB),
                    in1=sc_bc,
                    op=AluOpType.mult,
                )
                nc.vector.tensor_tensor(
                    out=ws4[:, :, 512:1024].rearrange("p k (r b) -> p k r b", b=BB),
                    in0=lo_f[:].rearrange("p (k r b) -> p k r b", k=CK, b=BB),
                    in1=sc_bc,
                    op=AluOpType.mult,
                )

                for kl in range(CK):
                    k = c * CK + kl
                    lhsT = xt_sb[:, k * B : (k + 1) * B]
                    first = k == 0
                    last = k == KT - 1
                    nc.tensor.matmul(
                        psW[:, 0:512], lhsT, ws4[:, kl, 0:512], start=first, stop=last
                    )
                    nc.tensor.matmul(
                        psW[:, 512:1024], lhsT, ws4[:, kl, 512:1024],
                        start=first, stop=last,
                    )
                    nc.tensor.matmul(
                        psC[:], lhsT, sz_sb[:, k * BB : (k + 1) * BB],
                        start=first, stop=last,
                    )

            out_sb = opool.tile([B, 1024], f32)
            psC_sb = opool.tile([B, BB], f32)
            nc.scalar.copy(out=psC_sb[:], in_=psC[:])
            psC_bc = psC_sb[:].unsqueeze(1).broadcast_to((B, 32, BB))
            nc.vector.tensor_tensor(
                out=out_sb[:, 0:512].rearrange("p (r b) -> p r b", b=BB),
                in0=psW[:, 0:512].rearrange("p (r b) -> p r b", b=BB),
                in1=psC_bc,
                op=AluOpType.subtract,
            )
            nc.vector.tensor_tensor(
                out=out_sb[:, 512:1024].rearrange("p (r b) -> p r b", b=BB),
                in0=psW[:, 512:1024].rearrange("p (r b) -> p r b", b=BB),
                in1=psC_bc,
                op=AluOpType.subtract,
            )
            nc.vector.tensor_tensor(
                out=out_sb[:], in0=out_sb[:], in1=bias_sb[:], op=AluOpType.add
            )
            nc.sync.dma_start(out=y_d[:], in_=out_sb[:])

    nc.compile()
    return nc


def _get_nc():
    if "nc" not in _CACHE:
        _CACHE["nc"] = _build_nc()
    return _CACHE["nc"]


def _prep_inputs(x, W_q, scale, zero, bias):
    """Host-side shard + layout prep (dtype narrowing / bit repack / transposes)."""
    xt = (
        x.T.reshape(KT, 128, B).transpose(1, 0, 2).reshape(128, KT * B)
    ).astype(ml_dtypes.bfloat16)  # (p, (k t))
    wq_u8 = W_q.astype(np.uint8)
    hi_u8 = (wq_u8 >> 4).astype(np.uint8)
    lo_u8 = (wq_u8 & 0xF).astype(np.uint8)
    sz_full = (scale.astype(np.float64) * zero.astype(np.float64)).astype(np.float32)

    def wlayout(arr_m):
        # arr_m: (32, NGC) one core's nibble slice -> [chunk, p, (k_local, r, b)]
        a = arr_m.reshape(32, BB, IN)          # (r, b, in)
        a = a.transpose(2, 0, 1)               # (in, r, b): col = r*16+b
        a = a.reshape(NCH, CK, 128, 512)       # (c, kl, p, rb)
        a = a.transpose(0, 2, 1, 3)            # (c, p, kl, rb)
        return np.ascontiguousarray(a.reshape(NCH, 128, CK * 512))

    in_maps = []
    for m in range(NCORES):
        sl = slice(m * NGC, (m + 1) * NGC)
        sc_m = (
            scale[0, sl]
            .reshape(BB, IN)
            .T.reshape(KT, 128, BB)
            .transpose(1, 0, 2)
            .reshape(128, KT * BB)
        ).astype(ml_dtypes.bfloat16)
        sz_m = (
            sz_full[0, sl]
            .reshape(BB, IN)
            .T.reshape(KT, 128, BB)
            .transpose(1, 0, 2)
            .reshape(128, KT * BB)
        ).astype(ml_dtypes.bfloat16)
        # out col c = h*512 + r*16 + b  <->  global out o = (h*32+r)*128 + m*16 + b
        bs_m = (
            bias.reshape(GS, 128)[:, m * BB : (m + 1) * BB]  # (gs, b)
            .reshape(1, 1024)
            .astype(np.float32)
        )
        in_maps.append(
            {
                "xt": xt,
                "hi": wlayout(hi_u8[:, sl]),
                "lo": wlayout(lo_u8[:, sl]),
                "sc": np.ascontiguousarray(sc_m),
                "sz": np.ascontiguousarray(sz_m),
                "bs": bs_m,
            }
        )
    return in_maps


def _gather(results):
    ybig = np.stack([results[m]["y"] for m in range(NCORES)], axis=1)  # (t, m, 1024)
    ybig = ybig.reshape(B, NCORES, 2, 32, BB)  # (t, m, h, r, b)
    return np.ascontiguousarray(
        ybig.transpose(0, 2, 3, 1, 4).reshape(B, OUT)
    )  # o = (h*32+r)*128 + m*16 + b


def run_on_hw(x, W_q, scale, zero, bias, trace=False, **trace_kw):
    """Returns (y_full, BassKernelResults)."""
    from concourse.bass_utils import run_bass_kernel_spmd

    nc = _get_nc()
    in_maps = _prep_inputs(x, W_q, scale, zero, bias)
    res = run_bass_kernel_spmd(
        nc, in_maps, list(range(NCORES)), trace=trace, **trace_kw
    )
    return _gather(res.results), res


def kernel(x, W_q, scale, zero, bias):
    y, _ = run_on_hw(x, W_q, scale, zero, bias, trace=False)
    return y

